# revision 46
# baseline (speedup 1.0000x reference)
"""BlockTucker fusion kernel for 8 Trainium2 NeuronCores.

Reference computation (per batch row b):
    h0 = x0 @ W0 + b0; h1 = x1 @ W1 + b1              # [B, 1600]
    per chunk c (20 chunks of 80):
        z[c,o] = sum_{s,t} h0c[s] Wb[c,o,s,t] h1c[t] + bb[c,o]
        z = signsqrt(z); z /= max(||z||_2, 1e-12)
    out = concat(z) @ Wout + bout                      # [B, 3000]

Strategy: pure data parallel over batch (1024 rows/core), bf16 compute.
The bilinear form is an outer-product matmul: per chunk, P^T[(s,t), b] =
h0[s,b]*h1[t,b] is built feature-major by DMA-replicating rows of H^T
across partitions and one DVE bf16 multiply per k-tile group; then
z^T[o,b] = sum_{st} WbT[(s,t),o] P^T[(s,t),b] over 50 k-tiles in PSUM.

The projection matmuls are software-pipelined INTO the bilinear chunk
loop (chunk c needs only h rows < 80(c+1)), so the DVE's P-construction
(the second-busiest engine) overlaps projection matmuls and the PE never
starves on P tiles.  h0/h1 round-trip through DRAM mirrors so the
replication DMAs can use partition-broadcast access patterns.
"""

import sys

sys.path.insert(0, "/opt/trn_rl_repo")

from contextlib import ExitStack

import numpy as np
import ml_dtypes

import concourse.bass as bass
import concourse.mybir as mybir
import concourse.tile as tile
from concourse import bacc
from concourse.bass_utils import run_bass_kernel_spmd

BF16 = mybir.dt.bfloat16
F32 = mybir.dt.float32
AF = mybir.ActivationFunctionType

B = 8192
D_IN = 2048
MM = 1600
CHUNKS = 20
CS = 80
D_OUT = 3000
N_CORES = 8
BL = B // N_CORES  # 1024 batch rows per core

K_IN = D_IN // 128  # 16 k-tiles for projections
MT_H = 13  # m-tiles for H (1600 -> 12x128 + 64)
MM_PAD = MT_H * 128  # 1664
KT_BIL = 50  # k-tiles per chunk for bilinear (6400/128)
MT_O = 24  # m-tiles for out (3000 -> 23x128 + 56)
D_OUT_PAD = MT_O * 128  # 3072
NH = BL // 512  # 2 free-dim halves of 512

# proj m-tile that must be complete before bilinear chunk c runs
_M_OF_C = [(80 * c + CS - 1) // 128 for c in range(CHUNKS)]


def _h_row_segments(mm0, nrows):
    """Split H rows [mm0, mm0+nrows) at 128-partition boundaries.

    Yields (row_off_in_block, p0, kt, n) for each contiguous segment.
    """
    segs = []
    a = mm0
    while a < mm0 + nrows:
        p0 = a % 128
        kt = a // 128
        n = min(128 - p0, mm0 + nrows - a)
        segs.append((a - mm0, p0, kt, n))
        a += n
    return segs


def _rep_dma(eng, dst, src_dram, mm0, nrows, reps):
    """dst[128, BL] <- partition p gets H row (mm0 + p//reps)."""
    p0, kt = mm0 % 128, mm0 // 128
    assert p0 + nrows <= 128
    sl = src_dram[p0 : p0 + nrows, kt, :]  # [nrows, BL]
    rep = sl.unsqueeze(1).broadcast_to([nrows, reps, BL])
    eng.dma_start(out=dst, in_=rep)


def _dup_dma(eng, dst, src_dram, mm0, nrows, reps):
    """dst[128, BL] <- partition p gets H row (mm0 + p % nrows)."""
    p0, kt = mm0 % 128, mm0 // 128
    assert p0 + nrows <= 128
    sl = src_dram[p0 : p0 + nrows, kt, :]  # [nrows, BL]
    rep = sl.unsqueeze(0).broadcast_to([reps, nrows, BL])
    eng.dma_start(out=dst, in_=rep)


def build_program():
    nc = bacc.Bacc("TRN2", target_bir_lowering=False, debug=False)

    # DRAM parameters (per-core shards / replicated weights)
    x0T = nc.dram_tensor("x0T", [D_IN, BL], BF16, kind="ExternalInput").ap()
    x1T = nc.dram_tensor("x1T", [D_IN, BL], BF16, kind="ExternalInput").ap()
    # w{0,1}q[p, mt, kt, m] = W[kt*128+p, mt*128+m] (m-tile-major, contiguous)
    w0q = nc.dram_tensor("w0q", [128, MT_H, K_IN, 128], BF16, kind="ExternalInput").ap()
    w1q = nc.dram_tensor("w1q", [128, MT_H, K_IN, 128], BF16, kind="ExternalInput").ap()
    # wbp2[c, half, p, lkt, o]; kt = 25*half + lkt
    wbp2 = nc.dram_tensor(
        "wbp2", [CHUNKS, 2, 128, 25, CS], BF16, kind="ExternalInput"
    ).ap()
    # woutq[p, mto, kt, m] = Woutp[kt*128+p, mto*128+m]
    woutq = nc.dram_tensor(
        "woutq", [128, MT_O, MT_H, 128], BF16, kind="ExternalInput"
    ).ap()
    b0c = nc.dram_tensor("b0c", [128, MT_H], F32, kind="ExternalInput").ap()
    b1c = nc.dram_tensor("b1c", [128, MT_H], F32, kind="ExternalInput").ap()
    bbT = nc.dram_tensor("bbT", [CS, CHUNKS], F32, kind="ExternalInput").ap()
    boutc = nc.dram_tensor("boutc", [128, MT_O], F32, kind="ExternalInput").ap()
    outT = nc.dram_tensor("outT", [D_OUT, BL], F32, kind="ExternalOutput").ap()
    rn_dram = nc.dram_tensor("rn_dram", [CHUNKS, BL], BF16).ap()
    h0_dram = nc.dram_tensor("h0_dram", [128, MT_H, BL], BF16).ap()
    h1_dram = nc.dram_tensor("h1_dram", [128, MT_H, BL], BF16).ap()

    with tile.TileContext(nc) as tc:
        _emit(
            tc, nc, x0T, x1T, w0q, w1q, wbp2, woutq, b0c, b1c, bbT, boutc,
            outT, rn_dram, h0_dram, h1_dram,
        )
    nc.compile()
    return nc


def _emit(
    tc, nc, x0T, x1T, w0q, w1q, wbp2, woutq, b0c, b1c, bbT, boutc, outT,
    rn_dram, h0_dram, h1_dram,
):
    ctx = ExitStack()
    with ctx:
        singles = ctx.enter_context(tc.tile_pool(name="singles", bufs=1))
        mm_psum = ctx.enter_context(tc.tile_pool(name="mm_psum", bufs=2, space="PSUM"))

        # constants / biases (gpsimd queue: tiny, keeps SP/Act free at start)
        b0s = singles.tile([128, MT_H], F32)
        nc.gpsimd.dma_start(out=b0s, in_=b0c)
        b1s = singles.tile([128, MT_H], F32)
        nc.gpsimd.dma_start(out=b1s, in_=b1c)
        bbs = singles.tile([CS, CHUNKS], F32)
        nc.gpsimd.dma_start(out=bbs, in_=bbT)
        bouts = singles.tile([128, MT_O], F32)
        nc.gpsimd.dma_start(out=bouts, in_=boutc)
        ones80 = singles.tile([CS, 1], BF16)
        nc.vector.memset(ones80, 1.0)
        ones180 = singles.tile([1, CS], BF16)
        nc.vector.memset(ones180, 1.0)
        eps1 = singles.tile([1, 1], F32)
        nc.vector.memset(eps1, 1e-24)

        # Z (normalized, repacked) for the final matmul: rows = c*80+o, padded
        # one tile per k-tile so the out-phase matmuls' waits are per-kt
        # (a single tile makes every out matmul wait on the LAST chunk)
        zbig = [
            singles.tile([128, BL], BF16, name=f"zbig{k}") for k in range(MT_H)
        ]
        nc.vector.memset(zbig[MT_H - 1][64:128, :], 0.0)

        wo_pool = ctx.enter_context(tc.tile_pool(name="wo", bufs=2))
        o_pool = ctx.enter_context(tc.tile_pool(name="opool", bufs=2))

        with ExitStack() as p13:
            xpool = p13.enter_context(tc.tile_pool(name="xpool", bufs=1))
            wproj = p13.enter_context(tc.tile_pool(name="wproj", bufs=2))
            hev_pool = p13.enter_context(tc.tile_pool(name="hev", bufs=2))
            dup_pool = p13.enter_context(tc.tile_pool(name="dup", bufs=2))
            rep_pool = p13.enter_context(tc.tile_pool(name="rep", bufs=3))
            p_pool = p13.enter_context(tc.tile_pool(name="ppool", bufs=3))
            wb_pool = p13.enter_context(tc.tile_pool(name="wbpool", bufs=2))
            post_pool = p13.enter_context(tc.tile_pool(name="post", bufs=1))
            zst_pool = p13.enter_context(tc.tile_pool(name="zst", bufs=2))
            nrm_pool = p13.enter_context(tc.tile_pool(name="nrm", bufs=1))
            rn_pool = p13.enter_context(tc.tile_pool(name="rn", bufs=1))
            rnb_pool = p13.enter_context(tc.tile_pool(name="rnb", bufs=2))
            zn_pool = p13.enter_context(tc.tile_pool(name="zn", bufs=2))
            zpsum_pool = p13.enter_context(
                tc.tile_pool(name="zpsum", bufs=2, space="PSUM")
            )
            nsq_psum = p13.enter_context(
                tc.tile_pool(name="nsq_psum", bufs=1, space="PSUM")
            )

            # x loads (SP queue), staggered by k-chunk; x1 first (h1 feeds the
            # chunk loop via a DRAM round-trip, so it is needed earliest).
            xs1 = xpool.tile([128, K_IN, BL], BF16)
            xr1 = x1T.rearrange("(kt p) b -> p kt b", p=128)
            for kt4 in range(0, K_IN, 2):
                nc.sync.dma_start(out=xs1[:, kt4 : kt4 + 2, :], in_=xr1[:, kt4 : kt4 + 2, :])
            xs0 = xpool.tile([128, K_IN, BL], BF16)
            xr0 = x0T.rearrange("(kt p) b -> p kt b", p=128)
            for kt4 in range(0, K_IN, 2):
                nc.sync.dma_start(out=xs0[:, kt4 : kt4 + 2, :], in_=xr0[:, kt4 : kt4 + 2, :])

            xs = (xs0, xs1)
            wq = (w0q, w1q)
            bias_s = (b0s, b1s)
            h_dram = (h0_dram, h1_dram)

            def emit_proj_mt(mt, inputs=(1, 0)):
                m0 = mt * 128
                mw = min(128, MM - m0)
                for inp in inputs:
                    wt = wproj.tile([128, K_IN, 128], BF16, tag="wt")
                    # weight loads on the Act queue: the SP queue carries the
                    # big x loads at start and must not delay wt(mt0)
                    nc.scalar.dma_start(
                        out=wt[:, :, :mw], in_=wq[inp][:, mt, :, :mw]
                    )
                    hev = hev_pool.tile([128, BL], BF16, tag="hev")
                    for h in range(NH):
                        ps = mm_psum.tile([128, 512], F32, tag="mmps")
                        for kt in range(K_IN):
                            nc.tensor.matmul(
                                out=ps[:mw, :],
                                lhsT=wt[:, kt, :mw],
                                rhs=xs[inp][:, kt, h * 512 : (h + 1) * 512],
                                start=(kt == 0),
                                stop=(kt == K_IN - 1),
                            )
                        nc.scalar.activation(
                            out=hev[:mw, h * 512 : (h + 1) * 512],
                            in_=ps[:mw, :],
                            func=AF.Identity,
                            bias=bias_s[inp][:mw, mt : mt + 1],
                            scale=1.0,
                        )
                    nc.sync.dma_start(out=h_dram[inp][:mw, mt, :], in_=hev[:mw, :])

            def emit_feeds(c):
                wbt0 = wb_pool.tile([128, 25, CS], BF16, tag="wbt")
                nc.scalar.dma_start(out=wbt0, in_=wbp2[c, 0])
                wbt1 = wb_pool.tile([128, 25, CS], BF16, tag="wbt")
                nc.scalar.dma_start(out=wbt1, in_=wbp2[c, 1])
                h1dup = dup_pool.tile([128, 5, BL], BF16, tag="h1dup")
                for j in range(5):
                    _dup_dma(nc.scalar, h1dup[:, j, :], h1_dram, 80 * c + 16 * j, 16, 8)
                return (wbt0, wbt1, h1dup)

            def emit_core(c, feeds, pool_assist=False):
                wbt01, h1dup = feeds[:2], feeds[2]
                zps = zpsum_pool.tile([CS, BL], F32, tag="zps")
                for i in range(10):
                    h0rep = rep_pool.tile([128, BL], BF16, tag="h0rep")
                    _rep_dma(nc.sync, h0rep, h0_dram, 80 * c + 8 * i, 8, 16)
                    pt5 = p_pool.tile([128, 5, BL], BF16, tag="pt5")
                    if pool_assist and i % 2 == 1:
                        # no projection matmuls left to fill PE time: DVE is
                        # the limiter, shift one k-slice of P to GPSIMD
                        nc.vector.tensor_mul(
                            pt5[:, 0:4, :],
                            h0rep.unsqueeze(1).broadcast_to([128, 4, BL]),
                            h1dup[:, 0:4, :],
                        )
                        nc.gpsimd.tensor_mul(
                            pt5[:, 4, :], h0rep, h1dup[:, 4, :]
                        )
                    else:
                        nc.vector.tensor_mul(
                            pt5,
                            h0rep.unsqueeze(1).broadcast_to([128, 5, BL]),
                            h1dup,
                        )
                    wbt = wbt01[i // 5]
                    lk0 = (i % 5) * 5
                    for j in range(5):
                        kt = 5 * i + j
                        for h in range(NH):
                            nc.tensor.matmul(
                                out=zps[:, h * 512 : (h + 1) * 512],
                                lhsT=wbt[:, lk0 + j, :],
                                rhs=pt5[:, j, h * 512 : (h + 1) * 512],
                                start=(kt == 0),
                                stop=(kt == KT_BIL - 1),
                            )
                return zps

            def emit_post1(c, zps, fast=False):
                # a = |z+bb|, g = sign(z+bb), s = sqrt(a), zst = s*g
                av = post_pool.tile([CS, BL], BF16, tag="av")
                nc.scalar.activation(
                    out=av, in_=zps, func=AF.Abs, bias=bbs[:, c : c + 1], scale=1.0
                )
                gv = post_pool.tile([CS, BL], BF16, tag="gv")
                nc.scalar.activation(
                    out=gv, in_=zps, func=AF.Sign, bias=bbs[:, c : c + 1], scale=1.0
                )
                sv = post_pool.tile([CS, BL], BF16, tag="sv")
                nc.scalar.activation(out=sv, in_=av, func=AF.Sqrt)
                zst = zst_pool.tile([CS, BL], BF16, tag="zst")
                # final chunk is latency-critical (gates the out phase): DVE
                # is idle then and ~4x faster than GPSIMD
                if fast:
                    nc.vector.tensor_mul(zst, sv, gv)
                else:
                    nc.gpsimd.tensor_mul(zst, sv, gv)
                return av, zst

            def emit_nsq(c, av):
                # nsq[b] = sum_o |z+bb| ( = ||signsqrt(z)||^2 ); nrm = sqrt
                nps = nsq_psum.tile([1, BL], F32, tag="nps")
                for h in range(NH):
                    nc.tensor.matmul(
                        out=nps[:, h * 512 : (h + 1) * 512],
                        lhsT=ones80,
                        rhs=av[:, h * 512 : (h + 1) * 512],
                        start=True,
                        stop=True,
                    )
                nrm = nrm_pool.tile([1, BL], F32, tag="nrm")
                nc.scalar.activation(out=nrm, in_=nps, func=AF.Sqrt, bias=eps1)
                return nrm

            def emit_rnchain(c, nrm, zst, halves=1):
                # halves=2 splits every step by batch half to shorten the
                # serial latency (used for the final chunk, which gates the
                # out-phase matmul via zbig)
                hw_ = BL // halves
                rn = rn_pool.tile([1, BL], F32, tag="rnt")
                rnb = rnb_pool.tile([CS, BL], BF16, tag="rnb")
                zn = zn_pool.tile([CS, BL], BF16, tag="znt")
                for hh in range(halves):
                    sl = slice(hh * hw_, (hh + 1) * hw_)
                    nc.vector.reciprocal_approx_fast(out=rn[:, sl], in_=nrm[:, sl])
                    nc.gpsimd.dma_start(out=rn_dram[c : c + 1, sl], in_=rn[:, sl])
                    nc.gpsimd.dma_start(
                        out=rnb[:, sl],
                        in_=rn_dram[c : c + 1, sl].partition_broadcast(CS),
                    )
                    nc.gpsimd.tensor_mul(zn[:, sl], zst[:, sl], rnb[:, sl])
                    for off, p0, kt, n in _h_row_segments(80 * c, CS):
                        nc.gpsimd.dma_start(
                            out=zbig[kt][p0 : p0 + n, sl], in_=zn[off : off + n, sl]
                        )

            # ---- pipelined projection + bilinear ----
            emit_proj_mt(0)
            emit_proj_mt(1)
            next_mt = 2
            feeds = emit_feeds(0)
            wot_pre = []
            prev = None  # (nrm, zst) of previous chunk
            for c in range(CHUNKS):
                if c == 3:
                    # prefetch the first out-matmul weight tiles (SP queue
                    # has slack mid-stream; tiles wait in wo_pool)
                    for mt in range(2):
                        wot = wo_pool.tile([128, MT_H, 128], BF16, tag="wot")
                        nc.sync.dma_start(out=wot, in_=woutq[:, mt, :, :])
                        wot_pre.append(wot)
                zps = emit_core(c, feeds, pool_assist=(c >= 16))
                av, zst = emit_post1(c, zps, fast=(c == CHUNKS - 1))
                if c + 1 < CHUNKS:
                    while next_mt <= min(_M_OF_C[c + 1] + 1, MT_H - 1):
                        emit_proj_mt(next_mt)
                        next_mt += 1
                nrm = emit_nsq(c, av)
                if prev is not None:
                    emit_rnchain(c - 1, *prev)
                prev = (nrm, zst)
                if c + 1 < CHUNKS:
                    feeds = emit_feeds(c + 1)
            # fast rn chain for the final chunk: it gates the out phase, so
            # skip the DRAM round-trip — broadcast rn across partitions with
            # a k=1 matmul into PSUM and run the segment DMAs on HWDGE
            c = CHUNKS - 1
            nrm, zst = prev
            rn_bf = rn_pool.tile([1, BL], BF16, tag="rnbf")
            with nc.allow_low_precision(reason="rn scale factor stored bf16"):
                nc.vector.reciprocal(rn_bf, nrm)
            rnb_ps = zpsum_pool.tile([CS, BL], F32, tag="zps")
            for h in range(NH):
                nc.tensor.matmul(
                    out=rnb_ps[:, h * 512 : (h + 1) * 512],
                    lhsT=ones180,
                    rhs=rn_bf[:, h * 512 : (h + 1) * 512],
                    start=True,
                    stop=True,
                )
            zn = zn_pool.tile([CS, BL], BF16, tag="znt")
            for h in range(NH):
                sl = slice(h * 512, (h + 1) * 512)
                nc.vector.tensor_mul(zn[:, sl], zst[:, sl], rnb_ps[:, sl])
            for off, p0, kt, n in _h_row_segments(80 * c, CS):
                nc.scalar.dma_start(
                    out=zbig[kt][p0 : p0 + n, :], in_=zn[off : off + n, :]
                )

            # ---- out^T = Wout^T-style matmul + bout ----
            # (emitted INSIDE the bilinear pool scope: closing those pools
            # would barrier the out phase behind the last chunk's rn chain)
            for mt in range(MT_O):
                m0 = mt * 128
                mw = min(128, D_OUT - m0)
                if mt < len(wot_pre):
                    wot = wot_pre[mt]
                else:
                    wot = wo_pool.tile([128, MT_H, 128], BF16, tag="wot")
                    nc.sync.dma_start(out=wot[:, :, :mw], in_=woutq[:, mt, :, :mw])
                for h in range(NH):
                    ps = mm_psum.tile([128, 512], F32, tag="mmps")
                    for kt in range(MT_H):
                        nc.tensor.matmul(
                            out=ps[:mw, :],
                            lhsT=wot[:, kt, :mw],
                            rhs=zbig[kt][:, h * 512 : (h + 1) * 512],
                            start=(kt == 0),
                            stop=(kt == MT_H - 1),
                        )
                    ot = o_pool.tile([128, 512], F32, tag="ot")
                    nc.scalar.activation(
                        out=ot[:mw, :],
                        in_=ps[:mw, :],
                        func=AF.Identity,
                        bias=bouts[:mw, mt : mt + 1],
                        scale=1.0,
                    )
                    nc.sync.dma_start(
                        out=outT[m0 : m0 + mw, h * 512 : (h + 1) * 512],
                        in_=ot[:mw, :],
                    )


_PROGRAM = None


def _get_program():
    global _PROGRAM
    if _PROGRAM is None:
        _PROGRAM = build_program()
    return _PROGRAM


def prep_weights(W0, b0, W1, b1, Wb, bb, Wout, bout):
    bf = ml_dtypes.bfloat16
    # w{0,1}q[p, mt, kt, m] = W[kt*128+p, mt*128+m]  (zero-padded m)
    def wq(W):
        Wp = np.zeros((D_IN, MM_PAD), np.float32)
        Wp[:, :MM] = W
        # [kt, p, mt, m] -> [p, mt, kt, m]
        r = Wp.reshape(K_IN, 128, MT_H, 128).transpose(1, 2, 0, 3)
        return np.ascontiguousarray(r, dtype=bf)

    w0 = wq(W0)
    w1 = wq(W1)
    # wbp[c, kt, p, o] = Wb[c, o, 8i + p//16, 16j + p%16], kt = 5i+j
    p = np.arange(128)
    wbp = np.empty((CHUNKS, KT_BIL, 128, CS), dtype=np.float32)
    for i in range(10):
        s_idx = 8 * i + p // 16
        for j in range(5):
            t_idx = 16 * j + p % 16
            wbp[:, 5 * i + j] = Wb[:, :, s_idx, t_idx].transpose(0, 2, 1)
    # wbp2[c, half, p, lkt, o]; kt = 25*half + lkt
    wbp2 = np.ascontiguousarray(
        wbp.reshape(CHUNKS, 2, 25, 128, CS).transpose(0, 1, 3, 2, 4), dtype=bf
    )
    # woutq[p, mto, kt, m] = Woutp[kt*128+p, mto*128+m]
    woutp = np.zeros((MM_PAD, D_OUT_PAD), np.float32)
    woutp[:MM, :D_OUT] = Wout
    woutq = np.ascontiguousarray(
        woutp.reshape(MT_H, 128, MT_O, 128).transpose(1, 2, 0, 3), dtype=bf
    )
    b0p = np.zeros(MM_PAD, np.float32)
    b0p[:MM] = b0
    b0c = np.ascontiguousarray(b0p.reshape(MT_H, 128).T)
    b1p = np.zeros(MM_PAD, np.float32)
    b1p[:MM] = b1
    b1c = np.ascontiguousarray(b1p.reshape(MT_H, 128).T)
    bbT = np.ascontiguousarray(np.asarray(bb, np.float32).T)
    boutp = np.zeros(D_OUT_PAD, np.float32)
    boutp[:D_OUT] = bout
    boutc = np.ascontiguousarray(boutp.reshape(MT_O, 128).T)
    return dict(
        w0q=w0, w1q=w1, wbp2=wbp2, woutq=woutq, b0c=b0c, b1c=b1c, bbT=bbT,
        boutc=boutc,
    )


def make_in_maps(x0, x1, weights):
    bf = ml_dtypes.bfloat16
    x0T = np.ascontiguousarray(np.asarray(x0).T, dtype=bf)
    x1T = np.ascontiguousarray(np.asarray(x1).T, dtype=bf)
    in_maps = []
    for r in range(N_CORES):
        sl = slice(r * BL, (r + 1) * BL)
        m = dict(weights)
        m["x0T"] = np.ascontiguousarray(x0T[:, sl])
        m["x1T"] = np.ascontiguousarray(x1T[:, sl])
        in_maps.append(m)
    return in_maps


def run(x0, x1, weights, **kwargs):
    nc = _get_program()
    in_maps = make_in_maps(x0, x1, weights)
    res = run_bass_kernel_spmd(nc, in_maps, core_ids=list(range(N_CORES)), **kwargs)
    out = np.empty((B, D_OUT), np.float32)
    for r in range(N_CORES):
        out[r * BL : (r + 1) * BL, :] = res.results[r]["outT"].T
    return out, res


def kernel(x0, x1, W0, b0, W1, b1, Wb, bb, Wout, bout):
    weights = prep_weights(W0, b0, W1, b1, Wb, bb, Wout, bout)
    out, _ = run(x0, x1, weights)
    return out


# ---- timed runner (no NTFF hook in this container: wall-clock the PJRT
# executable with device-resident inputs, minus dispatch overhead) ----

def _make_sharded_callable(nc, in_maps):
    import jax
    import numpy as _np
    from jax.sharding import Mesh, PartitionSpec, NamedSharding
    from jax.experimental.shard_map import shard_map
    from concourse import bass2jax as b2j
    from concourse import mybir as _mybir

    b2j.install_neuronx_cc_hook()
    n_cores = len(in_maps)
    partition_name = nc.partition_id_tensor.name if nc.partition_id_tensor else None
    in_names, out_names, out_avals, zero_outs = [], [], [], []
    for alloc in nc.m.functions[0].allocations:
        if not isinstance(alloc, _mybir.MemoryLocationSet):
            continue
        name = alloc.memorylocations[0].name
        if alloc.kind == "ExternalInput":
            if name != partition_name:
                in_names.append(name)
        elif alloc.kind == "ExternalOutput":
            shape = tuple(alloc.tensor_shape)
            dtype = _mybir.dt.np(alloc.dtype)
            out_names.append(name)
            out_avals.append(jax.core.ShapedArray(shape, dtype))
            zero_outs.append(_np.zeros(shape, dtype))
    n_params = len(in_names)
    in_names_all = list(in_names) + list(out_names)
    if partition_name is not None:
        in_names_all.append(partition_name)

    def _body(*args):
        operands = list(args)
        if partition_name is not None:
            operands.append(b2j.partition_id_tensor())
        outs = b2j._bass_exec_p.bind(
            *operands,
            out_avals=tuple(out_avals),
            in_names=tuple(in_names_all),
            out_names=tuple(out_names),
            lowering_input_output_aliases=(),
            sim_require_finite=True,
            sim_require_nnan=True,
            nc=nc,
        )
        return tuple(outs)

    devices = jax.devices()[:n_cores]
    mesh = Mesh(_np.asarray(devices), ("core",))
    spec = PartitionSpec("core")
    in_specs = (spec,) * (n_params + len(out_names))
    out_specs = (spec,) * len(out_names)
    n_outs = len(out_names)
    donate = tuple(range(n_params, n_params + n_outs))
    sharded = jax.jit(
        shard_map(_body, mesh=mesh, in_specs=in_specs, out_specs=out_specs,
                  check_rep=False),
        keep_unused=True,
        donate_argnums=donate,
    )
    sh = NamedSharding(mesh, spec)
    concat_in = [
        jax.device_put(
            _np.concatenate([_np.asarray(in_maps[c][n]) for c in range(n_cores)], 0), sh
        )
        for n in in_names
    ]
    state = {"outs": None}

    def _fresh_zeros():
        return [
            jax.device_put(_np.zeros((n_cores * z.shape[0], *z.shape[1:]), z.dtype), sh)
            for z in zero_outs
        ]

    def call():
        # outputs are donated back in as the next call's output seeds; the
        # kernel fully overwrites every output, so contents don't matter
        seeds = state["outs"] if state["outs"] is not None else _fresh_zeros()
        outs = sharded(*concat_in, *seeds)
        state["outs"] = list(outs)
        return outs
    return call, out_names, out_avals


def _make_tiny_callable():
    global _TINY3
    try:
        nc = _TINY3
    except NameError:
        nc = None
    if nc is None:
        nc = bacc.Bacc("TRN2", target_bir_lowering=False, debug=False)
        a = nc.dram_tensor("a", [128, 16], F32, kind="ExternalInput").ap()
        o = nc.dram_tensor("o", [128, 16], F32, kind="ExternalOutput").ap()
        with tile.TileContext(nc) as tc:
            with tc.tile_pool(name="p", bufs=1) as pool:
                t = pool.tile([128, 16], F32)
                nc.sync.dma_start(out=t, in_=a)
                nc.sync.dma_start(out=o, in_=t)
        nc.compile()
        _TINY3 = nc
    in_maps = [dict(a=np.zeros((128, 16), np.float32)) for _ in range(N_CORES)]
    call, _, _ = _make_sharded_callable(nc, in_maps)
    return call
